# revision 7
# baseline (speedup 1.0000x reference)
"""BitLinear Trainium2 kernel: y = (q @ unpack2bit(W).T) * (1/s) * group_scale.

Column-parallel over 8 NeuronCores: each core owns 1376 of the 11008 output
features.  Packed int32 weights stream from HBM untouched (45 MB total); the
2-bit fields are extracted on-chip.

Per-core device pipeline (j = packed-K index, 1024 rows after host transpose):
  1. sync (HWDGE) DMA loads a [128, 1376] int32 j-tile at line rate.
  2. One DVE copy converts it to int16 (values 0..255), then three fused
     tensor_scalar ops build shifted planes h_s = p >> 2s for s=1,2,3 via
     (p - bias) * 2^-2s with int16 output (round-half-even, bias chosen so
     rounding == truncation).  {p, h1, h2, h3} linearly span all four 2-bit
     fields: t_r = h_r - 4*h_{r+1}.
  3. The int16 planes feed TensorE directly *bitcast as fp16*: values
     0..255 reinterpret as exact subnormals v * 2^-24, which the PE
     multiplies exactly (no flush-to-zero).  Activation coefficients
     [q0, q1-4q0, q2-4q1, q3-4q2] are exact small fp16 ints; PSUM
     accumulates the exact integer dot product scaled by 2^-24.
  4. Epilogue applies (psum - S_q*2^-24) * (2^24 * group_scale / s).

Everything stays exactly representable, so the matmul is bit-exact integer
arithmetic; only the final scalings round, matching the reference within a
few ulps.
"""

import sys

sys.path.insert(0, "/opt/trn_rl_repo")

import numpy as np

import concourse.mybir as mybir
import concourse.tile as tile
from concourse import bacc
from concourse.bass_utils import run_bass_kernel_spmd

AluOp = mybir.AluOpType
f32 = mybir.dt.float32
f16 = mybir.dt.float16
i32 = mybir.dt.int32
i16 = mybir.dt.int16

B = 16          # batch rows
K = 4096        # in_features
M = 11008       # out_features
KP = K // 4     # packed K (int32 values holding 4 ternary weights each)
NCORES = 8
MS = M // NCORES            # 1376 out features per core
NJT = KP // 128             # 8 j-tiles per core
NDT = NJT // 2              # 4 double-width tiles (2 j-tiles side by side)
# PSUM free-dim chunks of the per-core output (one bank each)
CHUNKS = [(0, 512), (512, 512), (1024, MS - 1024)]

USE_DMA_CAST = False  # gpsimd casting DMA (int32->int16) vs sync DMA + DVE cast


def build_kernel_body(tc, pT_d, coef_d, sqv4_d, srecip4_d, gsv_d, out_d):
    nc = tc.nc
    with (
        tc.tile_pool(name="sbuf", bufs=2) as pool,
        tc.tile_pool(name="const", bufs=1) as cpool,
        tc.tile_pool(name="psum", bufs=1, space="PSUM") as psum_pool,
    ):
        W2 = 2 * MS  # double-width: two j-tiles side by side in free dim
        psums = [
            psum_pool.tile([B, ln], f32, tag=f"psum{ci}", name=f"psum{ci}")
            for ci, (_, ln) in enumerate(CHUNKS)
        ]

        # issue all weight loads first (halves, alternating HWDGE queues)
        # so DMA starts the moment the preamble ends
        p32s = []
        for dt in range(NDT):
            rows = slice(dt * 128, (dt + 1) * 128)
            p32 = pool.tile([128, W2], i32, tag="p32", bufs=3, name=f"p32_{dt}")
            eng = nc.sync if dt % 2 == 0 else nc.scalar
            eng.dma_start(p32[:, :MS], pT_d[rows, :MS])
            eng.dma_start(p32[:, MS:], pT_d[rows, MS:])
            p32s.append(p32)

        # Activation coefficients, preloaded once: [j128, jt*64 + r*16 + b]
        coef_sb = cpool.tile([128, NJT * 64], f16, tag="coef")
        nc.sync.dma_start(coef_sb[:], coef_d[:])

        for dt in range(NDT):
            p32 = p32s[dt]
            p16 = pool.tile([128, W2], i16, tag="p16")
            # shifted planes h_s = p >> 2s via exact fp32 arith + int16
            # round-half-even output (bias keeps every case below .5)
            h1 = pool.tile([128, W2], i16, tag="h1")
            h2 = pool.tile([128, W2], i16, tag="h2")
            h3 = pool.tile([128, W2], i16, tag="h3")
            # first tile: split ops per half so compute starts on the first
            # half-load; later tiles: full-width ops (fewer instructions)
            pieces = (
                (slice(0, MS), slice(MS, W2)) if dt == 0 else (slice(0, W2),)
            )
            for pc in pieces:
                nc.vector.tensor_copy(p16[:, pc], p32[:, pc])
                nc.vector.tensor_scalar(
                    h1[:, pc], p16[:, pc], 1.875, 0.25, AluOp.subtract, AluOp.mult
                )
                nc.vector.tensor_scalar(
                    h2[:, pc], p16[:, pc], 7.5, 0.0625, AluOp.subtract, AluOp.mult
                )
                nc.vector.tensor_scalar(
                    h3[:, pc], p16[:, pc], 31.5, 0.015625, AluOp.subtract, AluOp.mult
                )

            for side in range(2):
                jt = 2 * dt + side
                for r, plane in enumerate((p16, h1, h2, h3)):
                    lhsT = coef_sb[:, jt * 64 + r * 16 : jt * 64 + (r + 1) * 16]
                    for ci, (off, ln) in enumerate(CHUNKS):
                        nc.tensor.matmul(
                            psums[ci][:],
                            lhsT,
                            plane[
                                :, side * MS + off : side * MS + off + ln
                            ].bitcast(f16),
                            start=(dt == 0 and side == 0 and r == 0),
                            stop=(dt == NDT - 1 and side == 1 and r == 3),
                        )

        # epilogue: out = (psum - 4*S_q) * (gs / (4*s))
        sqv4 = cpool.tile([B, 1], f32, tag="sqv4")
        nc.sync.dma_start(sqv4[:], sqv4_d[:])
        srecip4 = cpool.tile([B, 1], f32, tag="srecip4")
        nc.sync.dma_start(srecip4[:], srecip4_d[:])
        gsv = cpool.tile([B, 1], f32, tag="gsv")
        nc.sync.dma_start(gsv[:], gsv_d[:])
        alpha = cpool.tile([B, 1], f32, tag="alpha")
        nc.vector.tensor_tensor(alpha[:], srecip4[:], gsv[:], AluOp.mult)

        for ci, (off, ln) in enumerate(CHUNKS):
            osb = pool.tile([B, ln], f32, tag=f"osb{ci}", name=f"osb{ci}")
            nc.vector.tensor_scalar(
                osb[:], psums[ci][:], sqv4[:], alpha[:], AluOp.subtract, AluOp.mult
            )
            nc.sync.dma_start(out_d[:, off : off + ln], osb[:])


def build_nc():
    nc = bacc.Bacc("TRN2", target_bir_lowering=False)
    pT_d = nc.dram_tensor("pT", [KP // 2, 2 * MS], i32, kind="ExternalInput")
    coef_d = nc.dram_tensor("coef", [128, NJT * 64], f16, kind="ExternalInput")
    sqv4_d = nc.dram_tensor("sqv4", [B, 1], f32, kind="ExternalInput")
    srecip4_d = nc.dram_tensor("srecip4", [B, 1], f32, kind="ExternalInput")
    gsv_d = nc.dram_tensor("gsv", [B, 1], f32, kind="ExternalInput")
    out_d = nc.dram_tensor("out", [B, MS], f32, kind="ExternalOutput")
    with tile.TileContext(nc) as tc:
        build_kernel_body(tc, pT_d, coef_d, sqv4_d, srecip4_d, gsv_d, out_d)
    nc.compile()
    return nc


def prepare_inputs(input, weight_packed, weight_scale):
    """Host-side shard/layout prep. Returns per-core input maps."""
    inp = np.asarray(input, dtype=np.float32)
    wp = np.asarray(weight_packed, dtype=np.int32)
    ws = np.asarray(weight_scale, dtype=np.float32)

    # activation quantization (matches reference: f32, round-half-even)
    amax = np.maximum(np.max(np.abs(inp), axis=-1, keepdims=True), np.float32(1e-5))
    s = np.float32(127.0) / amax                          # [B,1] f32
    q = np.clip(np.round(inp * s), -128.0, 127.0).astype(np.float32)  # [B,K]

    # coefficients for basis planes [P=p, h1=p>>2, h2=p>>4, h3=p>>6]
    # (fed to PE as fp16 subnormals = v * 2^-24):
    # sum_r q_r t_r = P*q0 + h1*(q1-4q0) + h2*(q2-4q1) + h3*(q3-4q2)
    # (all coefficient values are ints <= 635, exact in fp16)
    qr = q.reshape(B, KP, 4)
    c = np.empty((B, KP, 4), dtype=np.float32)
    c[..., 0] = qr[..., 0]
    c[..., 1] = qr[..., 1] - 4.0 * qr[..., 0]
    c[..., 2] = qr[..., 2] - 4.0 * qr[..., 1]
    c[..., 3] = qr[..., 3] - 4.0 * qr[..., 2]
    # -> SBUF layout [j128, jt*64 + r*16 + b]
    coefT = c.transpose(1, 2, 0).reshape(NJT, 128, 64)
    coef_sb = np.ascontiguousarray(
        coefT.transpose(1, 0, 2).reshape(128, NJT * 64)
    ).astype(np.float16)

    sqv4 = (q.sum(axis=-1, keepdims=True) * np.float32(2.0**-24)).astype(
        np.float32
    )  # S_q * 2^-24, exact
    srecip4 = (np.float32(2.0**24) / s).astype(np.float32)  # 2^24 / s

    in_maps = []
    for core in range(NCORES):
        m0 = core * MS
        # [KP, MS] -> double-width [KP/2, 2*MS]: row p of block dt holds
        # j = dt*256+p (cols 0:MS) and j = dt*256+128+p (cols MS:2*MS)
        pT_core = np.ascontiguousarray(
            wp[m0 : m0 + MS]
            .T.reshape(NJT // 2, 2, 128, MS)
            .transpose(0, 2, 1, 3)
            .reshape(KP // 2, 2 * MS)
        )
        gs = ws[(m0 // (M // ws.shape[0]))]
        in_maps.append(
            {
                "pT": pT_core,
                "coef": coef_sb,
                "sqv4": sqv4,
                "srecip4": srecip4,
                "gsv": np.full((B, 1), gs, dtype=np.float32),
            }
        )
    return in_maps


_NC_CACHE = {}


def run(input, weight_packed, weight_scale, trace=False):
    if "nc" not in _NC_CACHE:
        _NC_CACHE["nc"] = build_nc()
    nc = _NC_CACHE["nc"]
    in_maps = prepare_inputs(input, weight_packed, weight_scale)
    res = run_bass_kernel_spmd(nc, in_maps, core_ids=list(range(NCORES)), trace=trace)
    out = np.concatenate([r["out"] for r in res.results], axis=1)
    return out, res


def kernel(**inputs):
    out, _ = run(
        inputs["input"], inputs["weight_packed"], inputs["weight_scale"], trace=False
    )
    return out
